# revision 6
# baseline (speedup 1.0000x reference)
"""Trainium2 Bass kernel for the DiffusionProcess problem.

Strategy (hardcoded for B=2048, R=512, Z=256, H=512, T=16, 8 cores):
  - Data parallel: batch sharded 8 x 256, MLP weights replicated.
  - Feature-major layout on device: activations stored [feature, batch]
    so matmuls are out[M,N] = W[K,M].T @ x[K,N] with K,M tiles of 128 and
    N = 256 (the per-core batch); biases are per-partition columns.
  - Matmuls run in float32r (TF32) at ~1.3 PE cycles/row.
  - r @ W0[Z:] is step-invariant -> computed once before the scan.
  - temb_t @ W0 + b0 is batch-invariant -> precomputed for all 16 steps
    as [H, 16] columns, used as per-partition bias.
  - Step-boundary retiming: y = z + sqrt_dt*eps + dt*bo is precomputed
    off the critical path (eps is an input, known ahead), so between the
    last Wo matmul of step t and the first Wz matmul of step t+1 there
    is only ONE fused DVE op: z' = dt*psum + y. The reference's mu is
    reconstructed off-path as mu = z' - sqrt_dt*eps.
  - Matmul orders tuned per stage so each stage's first-needed psum
    group finishes early enough for its DVE/ACT chain to hide under the
    remaining matmuls (keeps the PE dense -> HAM stays at full clock).
  - Host pre-relayouts inputs so every DMA is one contiguous run per
    partition; streaming DMAs (eps in, zs/mus out) ride gpsimd SWDGE
    queues, weights ride the sync HWDGE queue.
"""

import sys

if "/opt/trn_rl_repo" not in sys.path:
    sys.path.insert(0, "/opt/trn_rl_repo")

import numpy as np

B, R, Z, H = 2048, 512, 256, 512
ZR = Z + R
T = 16
NC = 8
BS = B // NC          # 256 batch per core
DT = 1.0 / T
SQDT = DT ** 0.5
P = 128
KZ = Z // P           # 2
KR = R // P           # 4
KH = H // P           # 4
MH = H // P           # 4
MZ = Z // P           # 2
NF = ZR // P          # 6

_CACHE = {}


def _build():
    import concourse.bacc as bacc
    import concourse.tile as tile
    from concourse import mybir

    F32 = mybir.dt.float32
    F32R = mybir.dt.float32r
    AF = mybir.ActivationFunctionType
    OP = mybir.AluOpType

    nc = bacc.Bacc("TRN2", target_bir_lowering=False, debug=False,
                   num_devices=NC)

    # ---- DRAM tensors (per-core views; weights replicated).
    # Merged layouts: [128, ktiles*width] with k-tiles side by side.
    d_wz = nc.dram_tensor("wzb", [P, KZ * H], F32R, kind="ExternalInput").ap()
    d_wr = nc.dram_tensor("wrb", [P, KR * H], F32R, kind="ExternalInput").ap()
    d_wh = nc.dram_tensor("whb", [P, KH * H], F32R, kind="ExternalInput").ap()
    d_wo = nc.dram_tensor("wob", [P, KH * Z], F32R, kind="ExternalInput").ap()
    d_wt = nc.dram_tensor("wt", [1, ZR], F32R, kind="ExternalInput").ap()
    d_ts = nc.dram_tensor("ts", [1, T], F32R, kind="ExternalInput").ap()
    d_bt = nc.dram_tensor("btb", [P, NF], F32, kind="ExternalInput").ap()
    d_b0 = nc.dram_tensor("b0b", [P, MH], F32, kind="ExternalInput").ap()
    d_bh = nc.dram_tensor("bhb", [P, MH], F32, kind="ExternalInput").ap()
    d_bo = nc.dram_tensor("bob", [P, MZ], F32, kind="ExternalInput").ap()
    d_rt = nc.dram_tensor("rtb", [P, KR * BS], F32R,
                          kind="ExternalInput").ap()
    d_z0 = nc.dram_tensor("z0b", [P, KZ * BS], F32R,
                          kind="ExternalInput").ap()
    d_eps = nc.dram_tensor("epsb", [T, P, KZ * BS], F32,
                           kind="ExternalInput").ap()
    d_zs = nc.dram_tensor("zsb", [T, P, KZ * BS], F32R,
                          kind="ExternalOutput").ap()
    d_mus = nc.dram_tensor("musb", [T, P, KZ * BS], F32,
                           kind="ExternalOutput").ap()

    with tile.TileContext(nc) as tc:
        with tc.tile_pool(name="w", bufs=1) as wp, \
             tc.tile_pool(name="v", bufs=1) as vp, \
             tc.tile_pool(name="act", bufs=1) as ap_, \
             tc.tile_pool(name="st", bufs=2) as sp, \
             tc.tile_pool(name="ps", bufs=1, space="PSUM") as pp:

            # ---- loads, critical-path first (sync = HWDGE queue) ----
            ts = vp.tile([1, T], F32R, tag="ts", name="ts")
            nc.sync.dma_start(ts[:], d_ts[:])
            wt = vp.tile([1, ZR], F32R, tag="wt", name="wt")
            nc.sync.dma_start(wt[:], d_wt[:])
            btb = vp.tile([P, NF], F32, tag="btb", name="btb")
            nc.sync.dma_start(btb[:], d_bt[:])
            b0b = vp.tile([P, MH], F32, tag="b0b", name="b0b")
            nc.sync.dma_start(b0b[:], d_b0[:])
            z0b = sp.tile([P, KZ * BS], F32R, tag="z", name="z_0")
            nc.sync.dma_start(z0b[:], d_z0[:])
            wzb = wp.tile([P, KZ * H], F32R, tag="wzb", name="wzb")
            nc.sync.dma_start(wzb[:], d_wz[:])
            wrb = wp.tile([P, KR * H], F32R, tag="wrb", name="wrb")
            nc.sync.dma_start(wrb[:], d_wr[:])
            rtb = wp.tile([P, KR * BS], F32R, tag="rtb", name="rtb")
            nc.sync.dma_start(rtb[:], d_rt[:])
            bhb = vp.tile([P, MH], F32, tag="bhb", name="bhb")
            nc.sync.dma_start(bhb[:], d_bh[:])
            bob = vp.tile([P, MZ], F32, tag="bob", name="bob")
            nc.sync.dma_start(bob[:], d_bo[:])
            whb = wp.tile([P, KH * H], F32R, tag="whb", name="whb")
            nc.scalar.dma_start(whb[:], d_wh[:])
            wob = wp.tile([P, KH * Z], F32R, tag="wob", name="wob")
            nc.scalar.dma_start(wob[:], d_wo[:])

            def wz(k, m):
                return wzb[:, k * H + m * P: k * H + (m + 1) * P]

            def wr_(k, m):
                return wrb[:, k * H + m * P: k * H + (m + 1) * P]

            def wh(k, m):
                return whb[:, k * H + m * P: k * H + (m + 1) * P]

            def wo(k, m):
                return wob[:, k * Z + m * P: k * Z + (m + 1) * P]

            def w0(f, m):           # W0 row-tile f (z feats then r feats)
                return wz(f, m) if f < KZ else wr_(f - KZ, m)

            def rt(k):
                return rtb[:, k * BS:(k + 1) * BS]

            # dt * bo as per-partition columns (folded into y)
            dtbo = vp.tile([P, MZ], F32, tag="dtbo", name="dtbo")
            nc.scalar.activation(dtbo[:], bob[:], AF.Copy, scale=DT)

            # ---- temb[f] = relu(Wt_f^T ts + bt_f) : [128, T] ----
            temb = [ap_.tile([P, T], F32R, tag=f"temb{f}", name=f"temb{f}")
                    for f in range(NF)]
            for f in range(NF):
                ps = pp.tile([P, T], F32, tag=f"pa{f % MH}", name=f"pt{f}")
                nc.tensor.matmul(ps[:], wt[0:1, f * P:(f + 1) * P], ts[:],
                                 start=True, stop=True)
                nc.scalar.activation(temb[f][:], ps[:], AF.Relu,
                                     bias=btb[:, f:f + 1])

            # ---- c[m][:, t] = (temb_t @ W0 + b0)[m-tile] : [128, T] ----
            c = [ap_.tile([P, T], F32, tag=f"c{m}", name=f"c{m}")
                 for m in range(MH)]
            for m in range(MH):
                ps = pp.tile([P, T], F32, tag=f"pb{m}", name=f"pc{m}")
                for f in range(NF):
                    nc.tensor.matmul(ps[:], w0(f, m), temb[f][:],
                                     start=(f == 0), stop=(f == NF - 1))
                nc.scalar.activation(c[m][:], ps[:], AF.Identity,
                                     bias=b0b[:, m:m + 1])

            # ---- rW[m] = (r @ W0[Z:]) tile, feature-major [128, BS] ----
            rwps = [pp.tile([P, BS], F32, tag=f"pa{m}", name=f"prw{m}")
                    for m in range(MH)]
            for k in range(KR):
                for m in range(MH):
                    nc.tensor.matmul(rwps[m][:], wr_(k, m), rt(k),
                                     start=(k == 0), stop=(k == KR - 1))
            rw = [ap_.tile([P, BS], F32, tag=f"rw{m}", name=f"rw{m}")
                  for m in range(MH)]
            for m in range(MH):
                nc.scalar.activation(rw[m][:], rwps[m][:], AF.Copy)

            # ---- the scan ----
            pending = []

            def flush_pending():
                while pending:
                    pt, p_zn, p_eps = pending.pop(0)
                    mub = sp.tile([P, KZ * BS], F32, tag="mu",
                                  name=f"mu_{pt}", bufs=2)
                    for m in range(MZ):
                        nc.vector.scalar_tensor_tensor(
                            mub[:, m * BS:(m + 1) * BS], p_eps[m], -SQDT,
                            p_zn[m][:].bitcast(F32),
                            op0=OP.mult, op1=OP.add)
                        nc.gpsimd.dma_start(
                            d_zs[pt, :, m * BS:(m + 1) * BS], p_zn[m][:])
                    nc.gpsimd.dma_start(d_mus[pt], mub[:])

            z = [z0b[:, k * BS:(k + 1) * BS] for k in range(KZ)]
            for t in range(T):
                epsb = sp.tile([P, KZ * BS], F32, tag="e", name=f"e_{t}",
                               bufs=4)
                nc.gpsimd.dma_start(epsb[:], d_eps[t])
                eps = [epsb[:, k * BS:(k + 1) * BS] for k in range(KZ)]

                # previous step's mu + output DMAs (emitted here so the
                # DVE ops land after this step's tmp/a chain in the queue)
                flush_pending()

                # stage A (m-outer so ps_a[0] completes early):
                # ps_a[m] = z @ Wz ; a = relu(ps_a + c_t + rW)
                ps_a = [pp.tile([P, BS], F32, tag=f"pa{m}",
                                name=f"pa{m}_{t}") for m in range(MH)]
                for m in range(MH):
                    for k in range(KZ):
                        nc.tensor.matmul(ps_a[m][:], wz(k, m), z[k],
                                         start=(k == 0),
                                         stop=(k == KZ - 1))
                a = []
                for m in range(MH):
                    tmp = sp.tile([P, BS], F32, tag=f"tmp{m}",
                                  name=f"tmp{m}_{t}", bufs=1)
                    nc.vector.scalar_tensor_tensor(
                        tmp[:], ps_a[m][:], c[m][:, t:t + 1], rw[m][:],
                        op0=OP.add, op1=OP.add)
                    at = sp.tile([P, BS], F32R, tag=f"a{m}",
                                 name=f"a{m}_{t}", bufs=1)
                    nc.vector.tensor_scalar_max(at[:], tmp[:], 0.0)
                    a.append(at)

                # stage B (k-outer; needs only a[0] to start)
                ps_b = [pp.tile([P, BS], F32, tag=f"pb{m}",
                                name=f"pb{m}_{t}") for m in range(MH)]
                for k in range(KH):
                    for m in range(MH):
                        nc.tensor.matmul(ps_b[m][:], wh(k, m), a[k][:],
                                         start=(k == 0),
                                         stop=(k == KH - 1))
                g1 = []
                for m in range(MH):
                    g = sp.tile([P, BS], F32R, tag=f"g1{m}",
                                name=f"g1{m}_{t}", bufs=1)
                    nc.scalar.activation(g[:], ps_b[m][:], AF.Identity,
                                         bias=bhb[:, m:m + 1])
                    g1.append(g)

                # y = z + sqdt*eps + dt*bo  (off critical path; emitted
                # here so the boundary DVE queue is zn -> tmp/a)
                y = []
                for m in range(MZ):
                    y0 = sp.tile([P, BS], F32, tag=f"y0{m}",
                                 name=f"y0{m}_{t}", bufs=1)
                    nc.vector.scalar_tensor_tensor(
                        y0[:], eps[m], SQDT, z[m].bitcast(F32),
                        op0=OP.mult, op1=OP.add)
                    yt = sp.tile([P, BS], F32, tag=f"y{m}",
                                 name=f"y{m}_{t}", bufs=1)
                    nc.vector.tensor_scalar_add(yt[:], y0[:],
                                                dtbo[:, m:m + 1])
                    y.append(yt)

                # stage C: g2 = g1 @ Wh + bh (k-outer)
                ps_c = [pp.tile([P, BS], F32, tag=f"pa{m}",
                                name=f"pcc{m}_{t}") for m in range(MH)]
                for k in range(KH):
                    for m in range(MH):
                        nc.tensor.matmul(ps_c[m][:], wh(k, m), g1[k][:],
                                         start=(k == 0),
                                         stop=(k == KH - 1))
                g2 = []
                for m in range(MH):
                    g = sp.tile([P, BS], F32R, tag=f"g2{m}",
                                name=f"g2{m}_{t}", bufs=1)
                    nc.scalar.activation(g[:], ps_c[m][:], AF.Identity,
                                         bias=bhb[:, m:m + 1])
                    g2.append(g)

                # stage D: s = g2 @ Wo. Interleaved order finishes m=0's
                # accumulation 2 matmuls before the end so the z' chain
                # hides under the tail.
                ps_d = [pp.tile([P, BS], F32, tag=f"pb{m}",
                                name=f"pd{m}_{t}") for m in range(MZ)]
                for k, m in [(0, 0), (1, 0), (0, 1), (2, 0),
                             (1, 1), (3, 0), (2, 1), (3, 1)]:
                    nc.tensor.matmul(ps_d[m][:], wo(k, m), g2[k][:],
                                     start=(k == 0), stop=(k == KH - 1))

                # z' = dt*s + y (the only op on the step boundary)
                z_new = []
                for m in range(MZ):
                    zn = sp.tile([P, BS], F32R, tag=f"zn{m}",
                                 name=f"zn{m}_{t}", bufs=2)
                    nc.vector.scalar_tensor_tensor(
                        zn[:], ps_d[m][:], DT, y[m][:],
                        op0=OP.mult, op1=OP.add)
                    z_new.append(zn)
                pending.append((t, z_new, eps))
                z = [z_new[0][:], z_new[1][:]]
            flush_pending()

    nc.compile()
    return nc


def _get_nc():
    if "nc" not in _CACHE:
        _CACHE["nc"] = _build()
    return _CACHE["nc"]


def _ktile_merge(x, ktiles):
    """[ktiles*128, W] -> [128, ktiles*W] with k-tiles side by side."""
    w = x.shape[-1]
    return np.ascontiguousarray(
        x.reshape(ktiles, P, w).transpose(1, 0, 2).reshape(P, ktiles * w))


def _in_maps(inputs):
    f32 = lambda x: np.ascontiguousarray(np.asarray(x, dtype=np.float32))
    r = f32(inputs["r"])
    noise0 = f32(inputs["noise0"])
    noise = f32(inputs["noise"])
    W0 = f32(inputs["W0"])
    b0 = f32(inputs["b0"])
    Wh = f32(inputs["Wh"])
    bh = f32(inputs["bh"])
    Wo = f32(inputs["Wo"])
    bo = f32(inputs["bo"])
    Wt = f32(inputs["Wt"])
    bt = f32(inputs["bt"])

    shared = {
        "wzb": _ktile_merge(W0[:Z], KZ),
        "wrb": _ktile_merge(W0[Z:], KR),
        "whb": _ktile_merge(Wh, KH),
        "wob": _ktile_merge(Wo, KH),
        "wt": Wt.reshape(1, ZR),
        "ts": (np.arange(1, T + 1, dtype=np.float32)
               * np.float32(DT)).reshape(1, T),
        "btb": np.ascontiguousarray(bt.reshape(NF, P).T),
        "b0b": np.ascontiguousarray(b0.reshape(MH, P).T),
        "bhb": np.ascontiguousarray(bh.reshape(MH, P).T),
        "bob": np.ascontiguousarray(bo.reshape(MZ, P).T),
    }
    rT = np.ascontiguousarray(r.T)                         # [R, B]
    z0T = np.ascontiguousarray(noise0.T)                   # [Z, B]
    epsT = np.ascontiguousarray(noise.transpose(0, 2, 1))  # [T, Z, B]
    maps = []
    for cix in range(NC):
        s = slice(cix * BS, (cix + 1) * BS)
        m = dict(shared)
        m["rtb"] = _ktile_merge(np.ascontiguousarray(rT[:, s]), KR)
        m["z0b"] = _ktile_merge(np.ascontiguousarray(z0T[:, s]), KZ)
        ec = np.ascontiguousarray(epsT[:, :, s])           # [T, Z, BS]
        m["epsb"] = np.ascontiguousarray(
            ec.reshape(T, KZ, P, BS).transpose(0, 2, 1, 3)
            .reshape(T, P, KZ * BS))
        maps.append(m)
    return maps, noise0


def _unmerge(x):
    """[T, 128, KZ*BS] device layout -> [T, BS, Z] batch-major."""
    return (x.reshape(T, P, KZ, BS).transpose(0, 3, 2, 1)
            .reshape(T, BS, Z))


def _run(inputs, **run_kwargs):
    from concourse.bass_utils import run_bass_kernel_spmd
    nc = _get_nc()
    maps, noise0 = _in_maps(inputs)
    res = run_bass_kernel_spmd(nc, maps, core_ids=list(range(NC)),
                               **run_kwargs)
    out = np.empty((3, T + 1, B, Z), np.float32)
    out[0, 0] = noise0
    out[1, 0] = 0.0
    out[2, 0] = 1.0
    out[2, 1:] = np.float32(SQDT)
    for cix in range(NC):
        s = slice(cix * BS, (cix + 1) * BS)
        out[0, 1:, s, :] = _unmerge(res.results[cix]["zsb"])
        out[1, 1:, s, :] = _unmerge(res.results[cix]["musb"])
    return out, res


def kernel(**inputs) -> np.ndarray:
    out, _ = _run(inputs)
    return out


# revision 7
# speedup vs baseline: 1.0029x; 1.0029x over previous
"""Trainium2 Bass kernel for the DiffusionProcess problem.

Strategy (hardcoded for B=2048, R=512, Z=256, H=512, T=16, 8 cores):
  - Data parallel: batch sharded 8 x 256, MLP weights replicated.
  - Feature-major layout on device: activations stored [feature, batch]
    so matmuls are out[M,N] = W[K,M].T @ x[K,N] with K,M tiles of 128 and
    N = 256 (the per-core batch); biases are per-partition columns.
  - Matmuls run in float32r (TF32) at ~1.3 PE cycles/row.
  - r @ W0[Z:] is step-invariant -> computed once before the scan.
  - temb_t @ W0 + b0 is batch-invariant -> precomputed for all 16 steps
    as [H, 16] columns, used as per-partition bias.
  - Step-boundary retiming: y = z + sqrt_dt*eps + dt*bo is precomputed
    off the critical path (eps is an input, known ahead), so between the
    last Wo matmul of step t and the first Wz matmul of step t+1 there
    is only ONE fused DVE op: z' = dt*psum + y. The reference's mu is
    reconstructed off-path as mu = z' - sqrt_dt*eps.
  - Matmul orders tuned per stage so each stage's first-needed psum
    group finishes early enough for its DVE/ACT chain to hide under the
    remaining matmuls (keeps the PE dense -> HAM stays at full clock).
  - Host pre-relayouts inputs so every DMA is one contiguous run per
    partition; streaming DMAs (eps in, zs/mus out) ride gpsimd SWDGE
    queues, weights ride the sync HWDGE queue.
"""

import sys

if "/opt/trn_rl_repo" not in sys.path:
    sys.path.insert(0, "/opt/trn_rl_repo")

import numpy as np

B, R, Z, H = 2048, 512, 256, 512
ZR = Z + R
T = 16
NC = 8
BS = B // NC          # 256 batch per core
DT = 1.0 / T
SQDT = DT ** 0.5
P = 128
KZ = Z // P           # 2
KR = R // P           # 4
KH = H // P           # 4
MH = H // P           # 4
MZ = Z // P           # 2
NF = ZR // P          # 6

_CACHE = {}


def _build():
    import concourse.bacc as bacc
    import concourse.tile as tile
    from concourse import mybir

    F32 = mybir.dt.float32
    F32R = mybir.dt.float32r
    AF = mybir.ActivationFunctionType
    OP = mybir.AluOpType

    nc = bacc.Bacc("TRN2", target_bir_lowering=False, debug=False,
                   num_devices=NC)

    # ---- DRAM tensors (per-core views; weights replicated).
    # Merged layouts: [128, ktiles*width] with k-tiles side by side.
    d_wz = nc.dram_tensor("wzb", [P, KZ * H], F32R, kind="ExternalInput").ap()
    d_wr = nc.dram_tensor("wrb", [P, KR * H], F32R, kind="ExternalInput").ap()
    d_wh = nc.dram_tensor("whb", [P, KH * H], F32R, kind="ExternalInput").ap()
    d_wo = nc.dram_tensor("wob", [P, KH * Z], F32R, kind="ExternalInput").ap()
    d_wt = nc.dram_tensor("wt", [1, ZR], F32R, kind="ExternalInput").ap()
    d_ts = nc.dram_tensor("ts", [1, T], F32R, kind="ExternalInput").ap()
    d_bt = nc.dram_tensor("btb", [P, NF], F32, kind="ExternalInput").ap()
    d_b0 = nc.dram_tensor("b0b", [P, MH], F32, kind="ExternalInput").ap()
    d_bh = nc.dram_tensor("bhb", [P, MH], F32, kind="ExternalInput").ap()
    d_bo = nc.dram_tensor("bob", [P, MZ], F32, kind="ExternalInput").ap()
    d_rt = nc.dram_tensor("rtb", [P, KR * BS], F32R,
                          kind="ExternalInput").ap()
    d_z0 = nc.dram_tensor("z0b", [P, KZ * BS], F32R,
                          kind="ExternalInput").ap()
    d_eps = nc.dram_tensor("epsb", [T, P, KZ * BS], F32,
                           kind="ExternalInput").ap()
    d_zs = nc.dram_tensor("zsb", [T, P, KZ * BS], F32R,
                          kind="ExternalOutput").ap()
    d_mus = nc.dram_tensor("musb", [T, P, KZ * BS], F32,
                           kind="ExternalOutput").ap()

    with tile.TileContext(nc) as tc:
        with tc.tile_pool(name="w", bufs=1) as wp, \
             tc.tile_pool(name="v", bufs=1) as vp, \
             tc.tile_pool(name="act", bufs=1) as ap_, \
             tc.tile_pool(name="st", bufs=2) as sp, \
             tc.tile_pool(name="ps", bufs=1, space="PSUM") as pp:

            # ---- loads, critical-path first (sync = HWDGE queue) ----
            ts = vp.tile([1, T], F32R, tag="ts", name="ts")
            nc.sync.dma_start(ts[:], d_ts[:])
            wt = vp.tile([1, ZR], F32R, tag="wt", name="wt")
            nc.sync.dma_start(wt[:], d_wt[:])
            btb = vp.tile([P, NF], F32, tag="btb", name="btb")
            nc.sync.dma_start(btb[:], d_bt[:])
            b0b = vp.tile([P, MH], F32, tag="b0b", name="b0b")
            nc.sync.dma_start(b0b[:], d_b0[:])
            z0b = sp.tile([P, KZ * BS], F32R, tag="z", name="z_0")
            nc.sync.dma_start(z0b[:], d_z0[:])
            wzb = wp.tile([P, KZ * H], F32R, tag="wzb", name="wzb")
            nc.sync.dma_start(wzb[:], d_wz[:])
            wrb = wp.tile([P, KR * H], F32R, tag="wrb", name="wrb")
            nc.sync.dma_start(wrb[:], d_wr[:])
            rtb = wp.tile([P, KR * BS], F32R, tag="rtb", name="rtb")
            nc.sync.dma_start(rtb[:], d_rt[:])
            bhb = vp.tile([P, MH], F32, tag="bhb", name="bhb")
            nc.sync.dma_start(bhb[:], d_bh[:])
            bob = vp.tile([P, MZ], F32, tag="bob", name="bob")
            nc.sync.dma_start(bob[:], d_bo[:])
            whb = wp.tile([P, KH * H], F32R, tag="whb", name="whb")
            nc.scalar.dma_start(whb[:], d_wh[:])
            wob = wp.tile([P, KH * Z], F32R, tag="wob", name="wob")
            nc.scalar.dma_start(wob[:], d_wo[:])

            def wz(k, m):
                return wzb[:, k * H + m * P: k * H + (m + 1) * P]

            def wr_(k, m):
                return wrb[:, k * H + m * P: k * H + (m + 1) * P]

            def wh(k, m):
                return whb[:, k * H + m * P: k * H + (m + 1) * P]

            def wo(k, m):
                return wob[:, k * Z + m * P: k * Z + (m + 1) * P]

            def w0(f, m):           # W0 row-tile f (z feats then r feats)
                return wz(f, m) if f < KZ else wr_(f - KZ, m)

            def rt(k):
                return rtb[:, k * BS:(k + 1) * BS]

            # dt * bo as per-partition columns (folded into y)
            dtbo = vp.tile([P, MZ], F32, tag="dtbo", name="dtbo")
            nc.scalar.activation(dtbo[:], bob[:], AF.Copy, scale=DT)

            # ---- temb[f] = relu(Wt_f^T ts + bt_f) : [128, T] ----
            temb = [ap_.tile([P, T], F32R, tag=f"temb{f}", name=f"temb{f}")
                    for f in range(NF)]
            for f in range(NF):
                ps = pp.tile([P, T], F32, tag=f"pa{f % MH}", name=f"pt{f}")
                nc.tensor.matmul(ps[:], wt[0:1, f * P:(f + 1) * P], ts[:],
                                 start=True, stop=True)
                nc.scalar.activation(temb[f][:], ps[:], AF.Relu,
                                     bias=btb[:, f:f + 1])

            # ---- c[m][:, t] = (temb_t @ W0 + b0)[m-tile] : [128, T] ----
            c = [ap_.tile([P, T], F32, tag=f"c{m}", name=f"c{m}")
                 for m in range(MH)]
            for m in range(MH):
                ps = pp.tile([P, T], F32, tag=f"pb{m}", name=f"pc{m}")
                for f in range(NF):
                    nc.tensor.matmul(ps[:], w0(f, m), temb[f][:],
                                     start=(f == 0), stop=(f == NF - 1))
                nc.scalar.activation(c[m][:], ps[:], AF.Identity,
                                     bias=b0b[:, m:m + 1])

            # ---- rW[m] = (r @ W0[Z:]) tile, feature-major [128, BS] ----
            rwps = [pp.tile([P, BS], F32, tag=f"pa{m}", name=f"prw{m}")
                    for m in range(MH)]
            for k in range(KR):
                for m in range(MH):
                    nc.tensor.matmul(rwps[m][:], wr_(k, m), rt(k),
                                     start=(k == 0), stop=(k == KR - 1))
            rw = [ap_.tile([P, BS], F32, tag=f"rw{m}", name=f"rw{m}")
                  for m in range(MH)]
            for m in range(MH):
                nc.scalar.activation(rw[m][:], rwps[m][:], AF.Copy)

            # ---- the scan ----
            pending = []

            def flush_pending():
                while pending:
                    pt, p_zn, p_eps = pending.pop(0)
                    mub = sp.tile([P, KZ * BS], F32, tag="mu",
                                  name=f"mu_{pt}", bufs=2)
                    for m in range(MZ):
                        nc.vector.scalar_tensor_tensor(
                            mub[:, m * BS:(m + 1) * BS], p_eps[m], -SQDT,
                            p_zn[m][:].bitcast(F32),
                            op0=OP.mult, op1=OP.add)
                        nc.gpsimd.dma_start(
                            d_zs[pt, :, m * BS:(m + 1) * BS], p_zn[m][:])
                    nc.gpsimd.dma_start(d_mus[pt], mub[:])

            z = [z0b[:, k * BS:(k + 1) * BS] for k in range(KZ)]
            for t in range(T):
                epsb = sp.tile([P, KZ * BS], F32, tag="e", name=f"e_{t}",
                               bufs=4)
                nc.gpsimd.dma_start(epsb[:], d_eps[t])
                eps = [epsb[:, k * BS:(k + 1) * BS] for k in range(KZ)]

                # stage A (m-outer so ps_a[0] completes early):
                # ps_a[m] = z @ Wz ; a = relu(ps_a + c_t + rW)
                ps_a = [pp.tile([P, BS], F32, tag=f"pa{m}",
                                name=f"pa{m}_{t}") for m in range(MH)]
                for m in range(MH):
                    for k in range(KZ):
                        nc.tensor.matmul(ps_a[m][:], wz(k, m), z[k],
                                         start=(k == 0),
                                         stop=(k == KZ - 1))
                a = []
                for m in range(MH):
                    tmp = sp.tile([P, BS], F32, tag=f"tmp{m}",
                                  name=f"tmp{m}_{t}", bufs=1)
                    nc.vector.scalar_tensor_tensor(
                        tmp[:], ps_a[m][:], c[m][:, t:t + 1], rw[m][:],
                        op0=OP.add, op1=OP.add)
                    at = sp.tile([P, BS], F32R, tag=f"a{m}",
                                 name=f"a{m}_{t}", bufs=1)
                    nc.vector.tensor_scalar_max(at[:], tmp[:], 0.0)
                    a.append(at)

                # previous step's mu + output DMAs (after the a-chain so
                # they don't delay it in the DVE queue)
                flush_pending()

                # stage B (k-outer; needs only a[0] to start)
                ps_b = [pp.tile([P, BS], F32, tag=f"pb{m}",
                                name=f"pb{m}_{t}") for m in range(MH)]
                for k in range(KH):
                    for m in range(MH):
                        nc.tensor.matmul(ps_b[m][:], wh(k, m), a[k][:],
                                         start=(k == 0),
                                         stop=(k == KH - 1))
                g1 = []
                for m in range(MH):
                    g = sp.tile([P, BS], F32R, tag=f"g1{m}",
                                name=f"g1{m}_{t}", bufs=1)
                    nc.scalar.activation(g[:], ps_b[m][:], AF.Identity,
                                         bias=bhb[:, m:m + 1])
                    g1.append(g)

                # y = z + sqdt*eps + dt*bo  (off critical path; emitted
                # here so the boundary DVE queue is zn -> tmp/a)
                y = []
                for m in range(MZ):
                    y0 = sp.tile([P, BS], F32, tag=f"y0{m}",
                                 name=f"y0{m}_{t}", bufs=1)
                    nc.vector.scalar_tensor_tensor(
                        y0[:], eps[m], SQDT, z[m].bitcast(F32),
                        op0=OP.mult, op1=OP.add)
                    yt = sp.tile([P, BS], F32, tag=f"y{m}",
                                 name=f"y{m}_{t}", bufs=1)
                    nc.vector.tensor_scalar_add(yt[:], y0[:],
                                                dtbo[:, m:m + 1])
                    y.append(yt)

                # stage C: g2 = g1 @ Wh + bh (k-outer)
                ps_c = [pp.tile([P, BS], F32, tag=f"pa{m}",
                                name=f"pcc{m}_{t}") for m in range(MH)]
                for k in range(KH):
                    for m in range(MH):
                        nc.tensor.matmul(ps_c[m][:], wh(k, m), g1[k][:],
                                         start=(k == 0),
                                         stop=(k == KH - 1))
                g2 = []
                for m in range(MH):
                    g = sp.tile([P, BS], F32R, tag=f"g2{m}",
                                name=f"g2{m}_{t}", bufs=1)
                    nc.scalar.activation(g[:], ps_c[m][:], AF.Identity,
                                         bias=bhb[:, m:m + 1])
                    g2.append(g)

                # stage D: s = g2 @ Wo. Interleaved order finishes m=0's
                # accumulation 2 matmuls before the end so the z' chain
                # hides under the tail.
                ps_d = [pp.tile([P, BS], F32, tag=f"pb{m}",
                                name=f"pd{m}_{t}") for m in range(MZ)]
                for k, m in [(0, 0), (1, 0), (0, 1), (2, 0),
                             (1, 1), (3, 0), (2, 1), (3, 1)]:
                    nc.tensor.matmul(ps_d[m][:], wo(k, m), g2[k][:],
                                     start=(k == 0), stop=(k == KH - 1))

                # z' = dt*s + y (the only op on the step boundary)
                z_new = []
                for m in range(MZ):
                    zn = sp.tile([P, BS], F32R, tag=f"zn{m}",
                                 name=f"zn{m}_{t}", bufs=2)
                    nc.vector.scalar_tensor_tensor(
                        zn[:], ps_d[m][:], DT, y[m][:],
                        op0=OP.mult, op1=OP.add)
                    z_new.append(zn)
                pending.append((t, z_new, eps))
                z = [z_new[0][:], z_new[1][:]]
            flush_pending()

    nc.compile()
    return nc


def _get_nc():
    if "nc" not in _CACHE:
        _CACHE["nc"] = _build()
    return _CACHE["nc"]


def _ktile_merge(x, ktiles):
    """[ktiles*128, W] -> [128, ktiles*W] with k-tiles side by side."""
    w = x.shape[-1]
    return np.ascontiguousarray(
        x.reshape(ktiles, P, w).transpose(1, 0, 2).reshape(P, ktiles * w))


def _in_maps(inputs):
    f32 = lambda x: np.ascontiguousarray(np.asarray(x, dtype=np.float32))
    r = f32(inputs["r"])
    noise0 = f32(inputs["noise0"])
    noise = f32(inputs["noise"])
    W0 = f32(inputs["W0"])
    b0 = f32(inputs["b0"])
    Wh = f32(inputs["Wh"])
    bh = f32(inputs["bh"])
    Wo = f32(inputs["Wo"])
    bo = f32(inputs["bo"])
    Wt = f32(inputs["Wt"])
    bt = f32(inputs["bt"])

    shared = {
        "wzb": _ktile_merge(W0[:Z], KZ),
        "wrb": _ktile_merge(W0[Z:], KR),
        "whb": _ktile_merge(Wh, KH),
        "wob": _ktile_merge(Wo, KH),
        "wt": Wt.reshape(1, ZR),
        "ts": (np.arange(1, T + 1, dtype=np.float32)
               * np.float32(DT)).reshape(1, T),
        "btb": np.ascontiguousarray(bt.reshape(NF, P).T),
        "b0b": np.ascontiguousarray(b0.reshape(MH, P).T),
        "bhb": np.ascontiguousarray(bh.reshape(MH, P).T),
        "bob": np.ascontiguousarray(bo.reshape(MZ, P).T),
    }
    rT = np.ascontiguousarray(r.T)                         # [R, B]
    z0T = np.ascontiguousarray(noise0.T)                   # [Z, B]
    epsT = np.ascontiguousarray(noise.transpose(0, 2, 1))  # [T, Z, B]
    maps = []
    for cix in range(NC):
        s = slice(cix * BS, (cix + 1) * BS)
        m = dict(shared)
        m["rtb"] = _ktile_merge(np.ascontiguousarray(rT[:, s]), KR)
        m["z0b"] = _ktile_merge(np.ascontiguousarray(z0T[:, s]), KZ)
        ec = np.ascontiguousarray(epsT[:, :, s])           # [T, Z, BS]
        m["epsb"] = np.ascontiguousarray(
            ec.reshape(T, KZ, P, BS).transpose(0, 2, 1, 3)
            .reshape(T, P, KZ * BS))
        maps.append(m)
    return maps, noise0


def _unmerge(x):
    """[T, 128, KZ*BS] device layout -> [T, BS, Z] batch-major."""
    return (x.reshape(T, P, KZ, BS).transpose(0, 3, 2, 1)
            .reshape(T, BS, Z))


def _run(inputs, **run_kwargs):
    from concourse.bass_utils import run_bass_kernel_spmd
    nc = _get_nc()
    maps, noise0 = _in_maps(inputs)
    res = run_bass_kernel_spmd(nc, maps, core_ids=list(range(NC)),
                               **run_kwargs)
    out = np.empty((3, T + 1, B, Z), np.float32)
    out[0, 0] = noise0
    out[1, 0] = 0.0
    out[2, 0] = 1.0
    out[2, 1:] = np.float32(SQDT)
    for cix in range(NC):
        s = slice(cix * BS, (cix + 1) * BS)
        out[0, 1:, s, :] = _unmerge(res.results[cix]["zsb"])
        out[1, 1:, s, :] = _unmerge(res.results[cix]["musb"])
    return out, res


def kernel(**inputs) -> np.ndarray:
    out, _ = _run(inputs)
    return out
